# revision 64
# baseline (speedup 1.0000x reference)
# Trainium2 Bass kernel for nn_Attention_57509612094021 (XCA-style channel attention).
#
# Sharding: pure data-parallel over batch (8 images -> 8 NeuronCores), no collectives.
# Per-core pipeline (one [192,128,128] image), all-fp16 storage / fp32 accumulate:
#   - stream 16-row blocks: 1x1 convs (fp16 matmuls on PE) -> zero-padded SBUF tiles,
#     depthwise 3x3 split: q,k slabs 0,1 on DVE (tensor_scalar 2x + tensor_tensor
#     add 2x), v slabs + qk slab 2 on PE as 9 diag-matmuls accumulating in PSUM,
#   - per-block batched DMA-xbar transposes of q|k (head-interleaved channel
#     order) -> stacked per-head [96,96] Gram accumulated in PSUM across the
#     whole image (its diagonal gives the L2 norms for free),
#   - rsqrt with one Newton step, temperature/row/col scaling, softmax (exp on
#     ACT), then proj is FUSED into attention: M_h = A_h @ Wp_h precomputed per
#     head, y = sum_h M_h^T.T @ v_h streamed in 512-px chunks straight to DRAM.
# Host side pre-permutes weights (head-interleaved qk, 64-aligned v slots) and
# builds the diagonal depthwise matrices, masks, and selector matrices.
import os
import sys
import time

sys.path.insert(0, "/opt/trn_rl_repo")
os.environ.setdefault("JAX_PLATFORMS", "axon")

import numpy as np
import ml_dtypes

import concourse.bass as bass
import concourse.tile as tile
from concourse import bacc, mybir
from concourse.bass_utils import run_bass_kernel_spmd

F32 = mybir.dt.float32
F32R = mybir.dt.float32r
BF16 = mybir.dt.bfloat16
F16 = mybir.dt.float16
AF = mybir.ActivationFunctionType
OP = mybir.AluOpType
bf16 = ml_dtypes.bfloat16
fp16 = np.float16

C = 192
O = 384  # 2C
H = W = 128
HW = H * W
HEADS = 4
CPH = 48
R = 16           # rows per block
NB = H // R      # 16 blocks
PXB = R * W      # 1024 pixels per block
TAPS = [(ky, kx) for ky in range(3) for kx in range(3)]
QK2_PE_TAPS = 9

# depthwise engine split: both v slabs all-PE (diag matmuls); qk slabs on DVE.


def build_nc():
    nc = bacc.Bacc("TRN2", target_bir_lowering=False, debug=False, num_devices=8)

    d_x = nc.dram_tensor("x", [C, HW], F16, kind="ExternalInput").ap()
    d_wqkT = nc.dram_tensor("wqkT", [C, O], F16, kind="ExternalInput").ap()
    d_wvT = nc.dram_tensor("wvT", [C, 256], F16, kind="ExternalInput").ap()
    d_wphead = nc.dram_tensor("wphead", [48, 4 * C], F16, kind="ExternalInput").ap()
    d_dwqk = nc.dram_tensor("dwqk", [O, 9], F32, kind="ExternalInput").ap()
    d_diagv0 = nc.dram_tensor("diagv0", [128, 9 * 128], F16, kind="ExternalInput").ap()
    d_diagv1 = nc.dram_tensor("diagv1", [128, 9 * 128], F16, kind="ExternalInput").ap()
    d_diagqk2 = nc.dram_tensor("diagqk2", [128, 9 * 128], F16, kind="ExternalInput").ap()
    d_gmask = nc.dram_tensor("gmask", [96, 4 * 96], F32, kind="ExternalInput").ap()
    d_tmmask = nc.dram_tensor("tmmask", [96, 4], F32, kind="ExternalInput").ap()
    d_idf32 = nc.dram_tensor("idf32", [128, 128], F32, kind="ExternalInput").ap()
    d_hsel = nc.dram_tensor("hsel", [4, 4 * 48], F32, kind="ExternalInput").ap()
    d_y = nc.dram_tensor("y", [C, HW], F32, kind="ExternalOutput").ap()

    with tile.TileContext(nc) as tc:
        # ---------------- pools ----------------
        consts = tc.alloc_tile_pool(name="consts", bufs=1)
        persist = tc.alloc_tile_pool(name="persist", bufs=1)
        xpool = tc.alloc_tile_pool(name="xpool", bufs=2)
        blkpool = tc.alloc_tile_pool(name="blkpool", bufs=2)
        qkbpool = tc.alloc_tile_pool(name="qkbpool", bufs=1)
        scpool = tc.alloc_tile_pool(name="scpool", bufs=1)
        pb_pool = tc.alloc_tile_pool(name="pbpool", bufs=1)
        ys_pool = tc.alloc_tile_pool(name="yspool", bufs=3)
        ps_gram = tc.alloc_tile_pool(name="ps_gram", bufs=1, space="PSUM")
        ps_conv = tc.alloc_tile_pool(name="ps_conv", bufs=5, space="PSUM")
        ps_dw = tc.alloc_tile_pool(name="ps_dw", bufs=2, space="PSUM")

        # ---------------- constants / weights ----------------
        wqkT_a = consts.tile([128, O], F16, tag="wqkT_a")
        wqkT_b = consts.tile([64, O], F16, tag="wqkT_b")
        wvT_a = consts.tile([128, 256], F16, tag="wvT_a")
        wvT_b = consts.tile([64, 256], F16, tag="wvT_b")
        wphead = consts.tile([48, 4, C], F16, tag="wphead")
        dwqk = [consts.tile([128, 9], F32, tag=f"dwqk{s}", name=f"dwqk{s}") for s in range(3)]
        diagv0 = consts.tile([128, 9, 128], F16, tag="diagv0")
        diagv1 = consts.tile([128, 9, 128], F16, tag="diagv1")
        diagqk2 = consts.tile([128, 9, 128], F16, tag="diagqk2")
        gmask = consts.tile([96, 4 * 96], F32, tag="gmask")
        tmmask = consts.tile([96, 4], F32, tag="tmmask")
        idf32 = consts.tile([128, 128], F32, tag="idf32")
        hsel = consts.tile([4, 4 * 48], F32, tag="hsel")

        nc.sync.dma_start(wqkT_a[:], d_wqkT[0:128, :])
        nc.sync.dma_start(wqkT_b[:], d_wqkT[128:192, :])
        nc.sync.dma_start(wvT_a[:], d_wvT[0:128, :])
        nc.sync.dma_start(wvT_b[:], d_wvT[128:192, :])
        nc.sync.dma_start(wphead[:], d_wphead[:].rearrange("p (h o) -> p h o", h=4))
        for s in range(3):
            nc.sync.dma_start(dwqk[s][:], d_dwqk[128 * s : 128 * (s + 1), :])
        nc.sync.dma_start(diagv0[:], d_diagv0[:].rearrange("p (t c) -> p t c", t=9))
        nc.sync.dma_start(diagv1[:], d_diagv1[:].rearrange("p (t c) -> p t c", t=9))
        nc.sync.dma_start(diagqk2[:], d_diagqk2[:].rearrange("p (t c) -> p t c", t=9))
        nc.sync.dma_start(gmask[:], d_gmask[:])
        nc.sync.dma_start(tmmask[:], d_tmmask[:])
        nc.sync.dma_start(idf32[:], d_idf32[:])
        nc.sync.dma_start(hsel[:], d_hsel[:])

        # preload ACT table sets used later (exp loads at use; sqrt preloaded here)
        actwarm = consts.tile([1, 8], F32, tag="actwarm")
        nc.vector.memset(actwarm[:], 1.0)
        nc.scalar.activation(actwarm[:], actwarm[:], AF.Sqrt)

        # ---------------- persistent tensors ----------------
        v_buf = [
            persist.tile([128, HW], F16, tag="v_buf0", name="v_buf0"),
            persist.tile([128, HW], F16, tag="v_buf1", name="v_buf1"),
        ]
        gram = ps_gram.tile([96, 4 * 96], F32, tag="gram")

        # padded pre-activation buffers, manual parity double-buffer
        # qk: 3 slabs; v: 2 slabs. A = base, B = 1-shifted copy (qk only).
        PADW = 132
        NPR = R + 2
        padqkA = [[persist.tile([128, NPR, PADW], F16, tag=f"pqA{s}{p}", name=f"pqA{s}{p}") for p in range(2)] for s in range(3)]
        padvA = [
            [persist.tile([128, NPR, PADW], F16, tag=f"pvA0{p}", name=f"pvA0{p}") for p in range(2)],
            [persist.tile([128, NPR, PADW], F16, tag=f"pvA1{p}", name=f"pvA1{p}") for p in range(2)],
        ]
        # zero the side columns once (cols 0,1,130,131 never written later)
        for s in range(3):
            for p in range(2):
                nc.vector.memset(padqkA[s][p][:, :, 0:2], 0.0)
                nc.vector.memset(padqkA[s][p][:, :, 130:132], 0.0)
        for s in range(2):
            for p in range(2):
                nc.vector.memset(padvA[s][p][:, :, 0:2], 0.0)
                nc.vector.memset(padvA[s][p][:, :, 130:132], 0.0)
        # zero halo rows used by first block (parity 0)
        for s in range(3):
            nc.vector.memset(padqkA[s][0][:, 0:1, :], 0.0)
        for s in range(2):
            nc.vector.memset(padvA[s][0][:, 0:1, :], 0.0)

        dwsc = [scpool.tile([128, PXB], F16, tag=f"dwsc{i}", name=f"dwsc{i}") for i in range(1)]
        dwtmp = scpool.tile([128, PXB], F16, tag="dwtmp", name="dwtmp")

        # ---------------- phase A: blocks ----------------
        for b in range(NB):
            par = b % 2
            r0 = b * R
            lo = max(r0 - 1, 0)
            hi = min(r0 + R, H - 1)
            nr = hi - lo + 1
            row_off = lo - (r0 - 1)  # 1 for b==0 else 0
            npx = nr * W

            if b == NB - 1:
                # zero the bottom halo row (stale from block b-2)
                for s in range(3):
                    nc.vector.memset(padqkA[s][par][:, R + 1 : R + 2, :], 0.0)
                for s in range(2):
                    nc.vector.memset(padvA[s][par][:, R + 1 : R + 2, :], 0.0)

            x16_a = xpool.tile([128, npx], F16, tag="x16_a")
            x16_b = xpool.tile([64, npx], F16, tag="x16_b")
            nc.sync.dma_start(x16_a[:], d_x[0:128, lo * W : (hi + 1) * W])
            nc.sync.dma_start(x16_b[:], d_x[128:192, lo * W : (hi + 1) * W])

            # conv chunk row split (bank-aligned: 512-elem boundaries)
            chunks = [(i, min(4, nr - i)) for i in range(0, nr, 4)]

            def conv_to_pad(wa, wb, mlo, mhi, dst, n_mpart, on_dve=False):
                # output channels [mlo:mhi) -> dst pad tile ([n_mpart, NPR, PADW])
                for (cr0, crn) in chunks:
                    ps = ps_conv.tile([128, 4, W], F32, tag="conv")
                    pss = ps[:n_mpart, :crn, :]
                    rhs_a = x16_a[:, cr0 * W : (cr0 + crn) * W]
                    rhs_b = x16_b[:, cr0 * W : (cr0 + crn) * W]
                    nc.tensor.matmul(pss, wa[:, mlo:mhi], rhs_a, start=True, stop=False)
                    nc.tensor.matmul(pss, wb[:, mlo:mhi], rhs_b, start=False, stop=True)
                    dstv = dst[:n_mpart, row_off + cr0 : row_off + cr0 + crn, 2 : 2 + W]
                    if on_dve:
                        nc.vector.tensor_copy(dstv, pss)
                    else:
                        nc.scalar.copy(dstv, pss)

            for s in range(3):
                conv_to_pad(wqkT_a, wqkT_b, 128 * s, 128 * (s + 1), padqkA[s][par], 128)
            conv_to_pad(wvT_a, wvT_b, 0, 128, padvA[0][par], 128)
            conv_to_pad(wvT_a, wvT_b, 128, 256, padvA[1][par], 128)

            def win(padA, t, r_lo, rn):
                # input rows for out pad rows [1+r_lo, 1+r_lo+rn) are
                # [ky+r_lo, ky+r_lo+rn); input col base is 1+kx.
                ky, kx = TAPS[t]
                return padA[:, ky + r_lo : ky + r_lo + rn, 1 + kx : 1 + kx + W]

            # DVE depthwise for qk slabs 0,1: tensor_scalar (2x any-alignment) +
            # tensor_tensor add (2x, aligned contiguous operands)
            qk_blk = []
            for s in range(2):
                acc_final = qkbpool.tile([128, PXB], F16, tag=f"qkblk{s}", name=f"qkblk{s}_{b}")
                qk_blk.append(acc_final)
                acc = dwsc[0][:, 0:PXB]
                for t in range(9):
                    w_ap = dwqk[s][:, t : t + 1]
                    w3 = win(padqkA[s][par], t, 0, R)
                    if t == 0:
                        nc.vector.tensor_scalar(
                            acc.rearrange("p (r c) -> p r c", r=R), w3, w_ap, None, OP.mult
                        )
                    else:
                        tmp = dwtmp[:, 0:PXB]
                        nc.vector.tensor_scalar(
                            tmp.rearrange("p (r c) -> p r c", r=R), w3, w_ap, None, OP.mult
                        )
                        # in-place accumulate; final tap lands in qk_blk
                        out = acc_final[:, 0:PXB] if t == 8 else acc
                        nc.vector.tensor_add(out, tmp, acc)

            # PE depthwise: both v slabs and qk slab 2 (all taps), 4-row psum chunks
            qk2 = qkbpool.tile([128, PXB], F16, tag="qkblk2", name=f"qkblk2_{b}")
            qk_blk.append(qk2)
            pe_dw = [
                (padvA[0][par], diagv0, v_buf[0][:, r0 * W : (r0 + R) * W], range(9)),
                (padvA[1][par], diagv1, v_buf[1][:, r0 * W : (r0 + R) * W], range(9)),
                (padqkA[2][par], diagqk2, qk2[:, 0:PXB], range(QK2_PE_TAPS)),
            ]
            for pad, diag, dst, taps in pe_dw:
                tl = list(taps)
                full = len(tl) == 9
                for ci in range(PXB // 512):
                    ps = ps_dw.tile([128, 4, W], F32, tag="pedw")
                    for j, ti in enumerate(tl):
                        ky, kx = TAPS[ti]
                        rhs = pad[:, ky + 4 * ci : ky + 4 * ci + 4, 1 + kx : 1 + kx + W]
                        nc.tensor.matmul(
                            ps, diag[:, ti, :], rhs,
                            start=(j == 0), stop=(j == len(tl) - 1),
                        )
                    dstv = (dst if full else qk2part)[:, 512 * ci : 512 * (ci + 1)]
                    nc.scalar.copy(dstv.rearrange("p (r c) -> p r c", r=4), ps)
            # remaining qk2 taps on DVE, chained from the PE partial
            if QK2_PE_TAPS < 9:
                prev = qk2part[:, 0:PXB]
                rem = list(range(QK2_PE_TAPS, 9))
                for j, t in enumerate(rem):
                    w_ap = dwqk[2][:, t : t + 1]
                    w3 = win(padqkA[2][par], t, 0, R)
                    cur = (qk2 if j == len(rem) - 1 else dwsc[j % 2])[:, 0:PXB]
                    nc.vector.tensor_scalar(
                        dwtmp[:, 0:PXB].rearrange("p (r c) -> p r c", r=R), w3, w_ap, None, OP.mult
                    )
                    nc.vector.tensor_add(cur, dwtmp[:, 0:PXB], prev)
                    prev = cur

            # transpose q|k block -> [px, 384] groups via DMA xbar (batched:
            # out[p, g, c] = in[c, 128g+p]) and accumulate gram
            NG = PXB // 128
            qkT = blkpool.tile([128, NG * O], F16, tag="qkT")
            qkTv = qkT[:].rearrange("p (g o) -> p g o", g=NG)
            for s in range(3):
                nc.sync.dma_start_transpose(
                    qkTv[:, :, 128 * s : 128 * (s + 1)], qk_blk[s][:, 0:PXB]
                )
            for g in range(NG):
                for h in range(HEADS):
                    sl = qkT[:, g * O + 96 * h : g * O + 96 * (h + 1)]
                    nc.tensor.matmul(
                        gram[:, 96 * h : 96 * (h + 1)], sl, sl,
                        start=(b == 0 and g == 0), stop=(b == NB - 1 and g == NG - 1),
                        skip_group_check=True,
                    )

        # ---------------- phase B ----------------
        ps_dw.release()
        ps_conv.release()
        ps_misc = tc.alloc_tile_pool(name="ps_misc", bufs=2, space="PSUM")

        gram_sb = pb_pool.tile([96, 4 * 96], F32, tag="gram_sb")
        nc.vector.tensor_copy(gram_sb[:], gram[:])
        msk = pb_pool.tile([96, 4 * 96], F32, tag="msk")
        nc.vector.tensor_mul(msk[:], gram_sb[:], gmask[:])
        ss = pb_pool.tile([96, 4], F32, tag="ss")
        nc.vector.tensor_reduce(
            ss[:], msk[:].rearrange("p (h n) -> p h n", h=4), mybir.AxisListType.X, OP.add
        )
        rs = pb_pool.tile([96, 4], F32, tag="rs")
        nc.scalar.activation(rs[:], ss[:], AF.Sqrt)
        nc.vector.reciprocal(rs[:], rs[:])
        # one Newton step: rs *= 1.5 - 0.5*ss*rs^2  (ACT sqrt has a loose ULP budget)
        nt = pb_pool.tile([96, 4], F32, tag="nt")
        nc.vector.tensor_mul(nt[:], ss[:], rs[:])
        nc.vector.tensor_mul(nt[:], nt[:], rs[:])
        nc.vector.tensor_scalar(nt[:], nt[:], -0.5, 1.5, OP.mult, OP.add)
        nc.vector.tensor_mul(rs[:], rs[:], nt[:])
        nc.vector.tensor_mul(rs[:], rs[:], tmmask[:])  # fold temperature into q rows

        # row form of rs: [4, 96]
        ps_t = ps_misc.tile([128, 128], F32, tag="ps_misc")
        nc.tensor.transpose(ps_t[:4, :96], rs[:], idf32[:96, :96])
        rs_row = pb_pool.tile([4, 96], F32, tag="rs_row")
        nc.vector.tensor_copy(rs_row[:], ps_t[:4, :96])

        # column-scale tensor via selector matmuls: cs[h][c,d] = rs_k[h][d]
        ps_cs = ps_misc.tile([48, 4 * 48], F32, tag="ps_misc")
        for h in range(HEADS):
            nc.tensor.matmul(
                ps_cs[:, 48 * h : 48 * (h + 1)], hsel[:, 48 * h : 48 * (h + 1)],
                rs_row[:, 48:96], start=True, stop=True,
            )

        # S = G_qk * rs_q*temp (rows) * rs_k (cols)
        S = pb_pool.tile([48, 4 * 48], F32, tag="S")
        for h in range(HEADS):
            nc.vector.tensor_scalar(
                S[:, 48 * h : 48 * (h + 1)],
                gram_sb[0:48, 96 * h + 48 : 96 * h + 96],
                rs[0:48, h : h + 1],
                None, OP.mult,
            )
        nc.vector.tensor_mul(S[:], S[:], ps_cs[:])
        P = pb_pool.tile([48, 4 * 48], F32, tag="P")
        nc.scalar.activation(P[:], S[:], AF.Exp)
        den = pb_pool.tile([48, 4], F32, tag="den")
        nc.vector.tensor_reduce(
            den[:], P[:].rearrange("p (h n) -> p h n", h=4), mybir.AxisListType.X, OP.add
        )
        nc.vector.reciprocal(den[:], den[:])
        A = pb_pool.tile([48, 4 * 48], F16, tag="A")
        for h in range(HEADS):
            nc.vector.tensor_scalar(
                A[:, 48 * h : 48 * (h + 1)], P[:, 48 * h : 48 * (h + 1)],
                den[:, h : h + 1], None, OP.mult,
            )
        # fused attn+proj weights: MhT[d, o] = sum_c A_h[c, d] * WpT[48h+c, o],
        # stored with head h at rows 64*(h%2), col block h//2 (matches v_buf)
        MhT = pb_pool.tile([128, 2, C], F16, tag="MhT")
        for h in range(HEADS):
            ps_m = ps_misc.tile([128, C], F32, tag="ps_m", name=f"ps_m{h}")
            rlo = 64 * (h % 2)
            nc.tensor.matmul(
                ps_m[rlo : rlo + 48, :], A[:, 48 * h : 48 * (h + 1)],
                wphead[:, h, :], start=True, stop=True,
            )
            nc.scalar.copy(MhT[rlo : rlo + 48, h // 2, :], ps_m[rlo : rlo + 48, :])

        # fused attn@proj @ v -> out, in 512-px chunks
        ps_misc.release()
        ps_gram.release()
        ps_o = tc.alloc_tile_pool(name="ps_o", bufs=2, space="PSUM")
        NCH = HW // 512
        for ci in range(NCH):
            px = ci * 512
            py_a = ps_o.tile([128, 512], F32, tag="py_a")
            py_b = ps_o.tile([64, 512], F32, tag="py_b")
            for h in range(HEADS):
                rlo = 64 * (h % 2)
                rhs = v_buf[h // 2][rlo : rlo + 48, px : px + 512]
                nc.tensor.matmul(
                    py_a[:], MhT[rlo : rlo + 48, h // 2, 0:128], rhs,
                    start=(h == 0), stop=(h == 3),
                )
                nc.tensor.matmul(
                    py_b[:], MhT[rlo : rlo + 48, h // 2, 128:192], rhs,
                    start=(h == 0), stop=(h == 3),
                )
            ys_a = ys_pool.tile([128, 512], F32, tag="ys_a")
            ys_b = ys_pool.tile([64, 512], F32, tag="ys_b")
            nc.scalar.copy(ys_a[:], py_a[:])
            nc.vector.tensor_copy(ys_b[:], py_b[:])
            nc.sync.dma_start(d_y[0:128, px : px + 512], ys_a[:])
            nc.sync.dma_start(d_y[128:192, px : px + 512], ys_b[:])

        ps_o.release()
        ys_pool.release()
        pb_pool.release()
        scpool.release()
        qkbpool.release()
        blkpool.release()
        xpool.release()
        persist.release()
        consts.release()

    nc.compile()
    return nc


# ---------------- host side ----------------
_CACHE = {}


def _prep_static(W_qk, W_qk_dw, W_v, W_v_dw, W_proj, temperature):
    # head-interleaved channel permutation for qk: [q_h|k_h] blocks of 96
    perm = np.zeros(O, np.int64)
    for h in range(HEADS):
        perm[96 * h : 96 * h + 48] = np.arange(48 * h, 48 * h + 48)
        perm[96 * h + 48 : 96 * h + 96] = 192 + np.arange(48 * h, 48 * h + 48)

    wqkT = np.ascontiguousarray(W_qk[:, :, 0, 0].T[:, perm]).astype(np.float32)
    dwqk = np.ascontiguousarray(W_qk_dw[:, 0].reshape(O, 9)[perm]).astype(np.float32)

    # v channels padded to 64-aligned head slots: new chan (s,r): head 2s+r//64,
    # within-head idx r%64 (<48 live, else dead/zero). 256 slots = 2 slabs x 128.
    live = np.zeros(256, np.bool_)
    src_ch = np.zeros(256, np.int64)
    for s in range(2):
        for j in range(2):
            h = 2 * s + j
            r = 128 * s + 64 * j
            live[r : r + 48] = True
            src_ch[r : r + 48] = 48 * h + np.arange(48)

    wvT_orig = W_v[:, :, 0, 0].T.astype(np.float32)   # [192 in, 192 out]
    wvT = np.zeros((C, 256), np.float32)
    wvT[:, live] = wvT_orig[:, src_ch[live]]

    dwv_orig = W_v_dw[:, 0].reshape(C, 9).astype(np.float32)
    dwv = np.zeros((256, 9), np.float32)
    dwv[live] = dwv_orig[src_ch[live]]

    diagv0 = np.zeros((128, 9, 128), np.float32)
    diagv1 = np.zeros((128, 9, 128), np.float32)
    for t in range(9):
        diagv0[np.arange(128), t, np.arange(128)] = dwv[0:128, t]
        diagv1[np.arange(128), t, np.arange(128)] = dwv[128:256, t]

    # proj weights per head: wphead[c, h, o] = WpT[48h+c, o]
    wprojT_orig = W_proj[:, :, 0, 0].T.astype(np.float32)  # [192 in, 192 out]
    wphead = np.zeros((48, 4, C), np.float32)
    for h in range(HEADS):
        wphead[:, h, :] = wprojT_orig[48 * h : 48 * (h + 1), :]

    diagqk2 = np.zeros((128, 9, 128), np.float32)
    for t in range(9):
        diagqk2[np.arange(128), t, np.arange(128)] = dwqk[256:384, t]

    gmask = np.zeros((96, 4 * 96), np.float32)
    for h in range(HEADS):
        gmask[np.arange(96), 96 * h + np.arange(96)] = 1.0

    temp = np.asarray(temperature).reshape(HEADS)
    tmmask = np.ones((96, 4), np.float32)
    tmmask[0:48, :] = temp[None, :]

    hsel = np.zeros((4, 4 * 48), np.float32)
    for h in range(HEADS):
        hsel[h, 48 * h : 48 * (h + 1)] = 1.0

    return {
        "wqkT": wqkT.astype(fp16),
        "wvT": wvT.astype(fp16),
        "wphead": wphead.reshape(48, 4 * C).astype(fp16),
        "dwqk": dwqk,
        "diagv0": diagv0.reshape(128, 9 * 128).astype(fp16),
        "diagv1": diagv1.reshape(128, 9 * 128).astype(fp16),
        "diagqk2": diagqk2.reshape(128, 9 * 128).astype(fp16),
        "gmask": gmask,
        "tmmask": tmmask,
        "idf32": np.eye(128, dtype=np.float32),
        "hsel": hsel,
    }


def kernel(x, W_qk, W_qk_dw, W_v, W_v_dw, W_proj, temperature):
    x = np.asarray(x, np.float32)
    b = x.shape[0]
    assert b == 8 and x.shape[1] == C

    if "nc" not in _CACHE:
        _CACHE["nc"] = build_nc()
    nc = _CACHE["nc"]

    static = _prep_static(
        np.asarray(W_qk), np.asarray(W_qk_dw), np.asarray(W_v),
        np.asarray(W_v_dw), np.asarray(W_proj), np.asarray(temperature),
    )
    in_maps = []
    for i in range(b):
        m = dict(static)
        m["x"] = np.ascontiguousarray(x[i].reshape(C, HW)).astype(fp16)
        in_maps.append(m)

    res = run_bass_kernel_spmd(nc, in_maps, core_ids=list(range(8)))
    y = np.stack([res.results[i]["y"].reshape(C, H, W) for i in range(8)])
    return y.astype(np.float32)


if __name__ == "__main__":
    t0 = time.time()
    nc = build_nc()
    print(f"build+compile: {time.time()-t0:.1f}s")
